# revision 4
# baseline (speedup 1.0000x reference)
import sys

for _p in ("/opt/trn_rl_repo", "/root/.axon_site/_ro/trn_rl_repo"):
    if _p not in sys.path:
        sys.path.insert(0, _p)

import numpy as np
import ml_dtypes

# Problem constants (nn_LocalConvolution): x [4,256,64,64] f32,
# weight [4,1,16,49,64,64] f32, K=7, pad=3, stride=1, dil=1.
# out[b, g*16+cc, y, x] = sum_k x_pad[b, g*16+cc, y+kh-3, x+kw-3] * w[b,0,cc,k,y,x]
#
# Sharding: 8 cores = 4 batches x 2 H-halves (32 rows each).
# Per-core layout: partition p = yo*16 + cc  (yo = y-octave 0..7 -> rows
# yo*4..yo*4+3 of the half;  cc = weight channel 0..15).  This keeps the
# weights UNREPLICATED across partitions (3.2MB/core instead of the 25.7MB
# a channel-major layout needs), at the cost of a 2.5x row-halo on x.
B, C, H, W = 4, 256, 64, 64
WC, K, PAD = 16, 7, 3
NCORES = 8
HHALF = H // 2          # 32 output rows per core
PART = 128
G = C // WC             # 16 groups sharing each weight channel
NYO = 8                 # y-octaves per half (4 rows each)
RPO = HHALF // NYO      # 4 rows per octave
XROWS = RPO + K - 1     # 10 input rows per octave (3 halo each side)
XCOLS = 72              # 64 + 3 left pad + 5 right pad (even alignment)
OUTF = G * RPO * W      # 4096 f32 per partition = all 8 PSUM banks
BANK = 512

# ops whose products are computed on GPSIMD instead of DVE (load balance
# experiment; DVE is the throughput bottleneck otherwise)
GP_OPS = {(2, 3), (3, 3), (4, 3), (5, 3)}

_BF16 = ml_dtypes.bfloat16
_cache = {}


def _build():
    import concourse.bacc as bacc
    import concourse.mybir as mybir
    import concourse.tile as tile

    nc = bacc.Bacc(None, target_bir_lowering=False)
    bf = mybir.dt.bfloat16

    xa_d = nc.dram_tensor("xa", (PART, G * XROWS * XCOLS), bf, kind="ExternalInput")
    xb_d = nc.dram_tensor("xb", (PART, G * XROWS * XCOLS), bf, kind="ExternalInput")
    wr_d = nc.dram_tensor("wr", (PART, K * K * RPO * W), bf, kind="ExternalInput")
    id_d = nc.dram_tensor("ident", (PART, PART), bf, kind="ExternalInput")
    out_d = nc.dram_tensor("out", (PART, OUTF), bf, kind="ExternalOutput")

    KHRW = K * RPO * W  # 1792 weight elems per kw-chunk per partition
    GH = G * XROWS * XCOLS // 2  # half of xa's free size

    with tile.TileContext(nc) as tc:
        with (
            tc.tile_pool(name="xpool", bufs=1) as xpool,
            tc.tile_pool(name="cpool", bufs=1) as cpool,
            tc.tile_pool(name="wpool", bufs=1) as wpool,
            tc.tile_pool(name="tpool", bufs=5) as tpool,
            tc.tile_pool(name="opool", bufs=1) as opool,
            tc.tile_pool(name="psum", bufs=1, space="PSUM") as ppool,
        ):
            # x loads ride the (otherwise idle) scalar engine's DMA queue so
            # they stream in parallel with the weight chunks on the sync queue
            xa_t = xpool.tile([PART, G, XROWS, XCOLS], bf, tag="xa")
            xb_t = xpool.tile([PART, G, XROWS, XCOLS], bf, tag="xb")
            nc.scalar.dma_start(xa_t[:, : G // 2], xa_d[:, :GH])
            nc.scalar.dma_start(xa_t[:, G // 2 :], xa_d[:, GH:])
            nc.scalar.dma_start(xb_t[:], xb_d[:])
            # per-kw weight chunks: [kh, 1(g-bcast), r, x]
            w_t = []
            for kw in range(K):
                wt = wpool.tile([PART, K, 1, RPO, W], bf, tag=f"w{kw}")
                nc.sync.dma_start(wt[:], wr_d[:, kw * KHRW : (kw + 1) * KHRW])
                w_t.append(wt)
            id_t = cpool.tile([PART, PART], bf, tag="id")
            nc.sync.dma_start(id_t[:], id_d[:])

            acc = [
                ppool.tile([PART, BANK], mybir.dt.float32, name=f"ps{j}", tag=f"ps{j}")
                for j in range(2 * RPO)
            ]
            ost = opool.tile([PART, OUTF], bf, tag="ost")

            for kw in range(K):
                if kw % 2 == 0:
                    xx, c0 = xa_t, kw
                else:
                    xx, c0 = xb_t, kw + 1
                for r in range(RPO):
                    # products for all 7 kh taps of this (kw, output-row r):
                    # tmp[p, kh, g, x] = x[p, g, r+kh, c0+x] * w[p, kw, kh, r, x]
                    # kw==0 is split into g-halves so compute can start after
                    # half of xa has landed
                    halves = (0, 1) if kw == 0 else (None,)
                    for hv in halves:
                        if hv is None:
                            gs, ge = 0, G
                        else:
                            gs, ge = hv * (G // 2), (hv + 1) * (G // 2)
                        ng = ge - gs
                        tmp = tpool.tile([PART, K, ng, W], bf, tag="tmp")
                        eng = nc.gpsimd if (kw, r) in GP_OPS else nc.vector
                        xap = xx[:, gs:ge, r : r + K, c0 : c0 + W].transpose(
                            [0, 2, 1, 3]
                        )
                        wap = w_t[kw][:, :, 0:1, r, :].broadcast_to((PART, K, ng, W))
                        eng.tensor_mul(tmp[:], xap, wap)
                        # PE accumulates each kh-plane into PSUM; bank j = 2r+h
                        for kh in range(K):
                            for h in range(2):
                                if hv is not None and h != hv:
                                    continue
                                gofs = 0 if hv is not None else h * 8
                                j = 2 * r + h
                                nc.tensor.matmul(
                                    acc[j][:],
                                    id_t[:],
                                    tmp[:, kh, gofs : gofs + 8, :],
                                    start=(kw == 0 and kh == 0),
                                    stop=(kw == K - 1 and kh == K - 1),
                                )
                    if kw == K - 1:
                        for h in range(2):
                            j = 2 * r + h
                            nc.scalar.copy(
                                ost[:, j * BANK : (j + 1) * BANK], acc[j][:]
                            )
                        nc.sync.dma_start(
                            out_d[:, r * 2 * BANK : (r + 1) * 2 * BANK],
                            ost[:, r * 2 * BANK : (r + 1) * 2 * BANK],
                        )

    _dedupe_ldweights(nc)
    nc.compile()
    return nc


def _dedupe_ldweights(nc):
    """All PE matmuls share one identity stationary; drop every InstLdweights
    after the first so the PE array keeps the loaded weights."""
    first_repr = None
    removed = 0
    for blk in nc.main_func.blocks:
        keep = []
        for inst in blk.instructions:
            if type(inst).__name__ == "InstLdweights":
                si = inst.sync_info
                clean = si is None or (not si.on_wait and not si.on_update)
                r = repr(inst.ins[0])
                if first_repr is None:
                    first_repr = r
                elif clean and r == first_repr:
                    removed += 1
                    continue
            keep.append(inst)
        blk.instructions[:] = keep
    return removed


def _prep_core(x, w, b, h):
    """Host-side shard prep for one core: pad/cast/re-layout x, slice w."""
    y0 = h * HHALF
    # padded half: rows y0-3 .. y0+34 (38), cols -3 .. 68 (72)
    xpad = np.zeros((C, HHALF + 2 * PAD, XCOLS), dtype=np.float32)
    rlo, rhi = y0 - PAD, y0 + HHALF + PAD
    slo, shi = max(rlo, 0), min(rhi, H)
    xpad[:, slo - rlo : shi - rlo, PAD : PAD + W] = x[b, :, slo:shi, :]
    # xa[p=(yo,cc), g, row, col] = xpad[g*16+cc, yo*4+row, col]
    v = np.lib.stride_tricks.sliding_window_view(xpad, XROWS, axis=1)
    v = v[:, ::RPO]  # [256, 8yo, 72col, 10row]
    v = v.reshape(G, WC, NYO, XCOLS, XROWS)
    xa = v.transpose(2, 1, 0, 4, 3).reshape(PART, G, XROWS, XCOLS)
    xb = np.zeros_like(xa)
    xb[:, :, :, 1:] = xa[:, :, :, :-1]
    # w[p=(yo,cc), kw, kh, r, x] = w[b,0,cc,kh*7+kw, yo*4+r, x]
    ws = w[b, 0, :, :, y0 : y0 + HHALF, :]  # [16cc, 49k, 32y, 64x]
    ws = ws.reshape(WC, K, K, NYO, RPO, W)  # [cc, kh, kw, yo, r, x]
    wr = ws.transpose(3, 0, 2, 1, 4, 5).reshape(PART, -1)  # [p, kw,kh,r,x]
    return (
        np.ascontiguousarray(xa).reshape(PART, -1).astype(_BF16),
        xb.reshape(PART, -1).astype(_BF16),
        np.ascontiguousarray(wr).astype(_BF16),
    )


def _unpack_core(o):
    """[128, 4096] bf16 -> [256, 32, 64] f32 for one core."""
    # o[p=(yo,cc)][j=(r,h)][8g, 64x]
    o = np.asarray(o, dtype=np.float32).reshape(NYO, WC, RPO, 2, 8, W)
    # channel = (h*8+g)*16+cc, row = yo*4+r
    return o.transpose(3, 4, 1, 0, 2, 5).reshape(C, HHALF, W)


def kernel(x: np.ndarray, weight: np.ndarray) -> np.ndarray:
    from concourse.bass_utils import run_bass_kernel_spmd

    if "nc" not in _cache:
        _cache["nc"] = _build()
    nc = _cache["nc"]

    in_maps = make_inmaps(x, weight)
    res = run_bass_kernel_spmd(nc, in_maps, list(range(NCORES)))

    out = np.empty((B, C, H, W), dtype=np.float32)
    for core in range(NCORES):
        b, h = core // 2, core % 2
        out[b, :, h * HHALF : (h + 1) * HHALF, :] = _unpack_core(
            res.results[core]["out"]
        )
    return out


def make_inmaps(x, weight):
    ident = np.eye(PART, dtype=_BF16)
    in_maps = []
    for core in range(NCORES):
        b, h = core // 2, core % 2
        xa, xb, wr = _prep_core(x, weight, b, h)
        in_maps.append({"xa": xa, "xb": xb, "wr": wr, "ident": ident})
    return in_maps


# revision 6
# speedup vs baseline: 1.2534x; 1.2534x over previous
import sys

for _p in ("/opt/trn_rl_repo", "/root/.axon_site/_ro/trn_rl_repo"):
    if _p not in sys.path:
        sys.path.insert(0, _p)

import numpy as np
import ml_dtypes

# Problem constants (nn_LocalConvolution): x [4,256,64,64] f32,
# weight [4,1,16,49,64,64] f32, K=7, pad=3, stride=1, dil=1.
# out[b, g*16+cc, y, x] = sum_k x_pad[b, g*16+cc, y+kh-3, x+kw-3] * w[b,0,cc,k,y,x]
#
# Sharding: 8 cores = 4 batches x 2 H-halves (32 rows each).
# Per-core layout: partition p = yo*16 + cc  (yo = y-octave 0..7 -> rows
# yo*4..yo*4+3 of the half;  cc = weight channel 0..15).  This keeps the
# weights UNREPLICATED across partitions (3.2MB/core instead of the 25.7MB
# a channel-major layout needs), at the cost of a 2.5x row-halo on x.
B, C, H, W = 4, 256, 64, 64
WC, K, PAD = 16, 7, 3
NCORES = 8
HHALF = H // 2          # 32 output rows per core
PART = 128
G = C // WC             # 16 groups sharing each weight channel
NYO = 8                 # y-octaves per half (4 rows each)
RPO = HHALF // NYO      # 4 rows per octave
XROWS = RPO + K - 1     # 10 input rows per octave (3 halo each side)
XCOLS = 72              # 64 + 3 left pad + 5 right pad (even alignment)
OUTF = G * RPO * W      # 4096 f32 per partition = all 8 PSUM banks
BANK = 512

# xa-based (even) kw first so xb's DMA has ~60us of slack to land
KW_ORDER = [0, 2, 4, 6, 1, 3, 5]

_BF16 = ml_dtypes.bfloat16
_cache = {}


def _build():
    import concourse.bacc as bacc
    import concourse.mybir as mybir
    import concourse.tile as tile

    nc = bacc.Bacc(None, target_bir_lowering=False)
    bf = mybir.dt.bfloat16

    xa_d = nc.dram_tensor("xa", (PART, G * XROWS * XCOLS), bf, kind="ExternalInput")
    xb_d = nc.dram_tensor("xb", (PART, G * XROWS * XCOLS), bf, kind="ExternalInput")
    wr_d = nc.dram_tensor("wr", (PART, K * K * RPO * W), bf, kind="ExternalInput")
    id_d = nc.dram_tensor("ident", (PART, PART), bf, kind="ExternalInput")
    out_d = nc.dram_tensor("out", (PART, OUTF), bf, kind="ExternalOutput")

    KHRW = K * RPO * W  # 1792 weight elems per kw-chunk per partition
    GH = G * XROWS * XCOLS // 2  # half of xa's free size

    with tile.TileContext(nc) as tc:
        with (
            tc.tile_pool(name="xpool", bufs=1) as xpool,
            tc.tile_pool(name="cpool", bufs=1) as cpool,
            tc.tile_pool(name="wpool", bufs=1) as wpool,
            tc.tile_pool(name="tpool", bufs=5) as tpool,
            tc.tile_pool(name="opool", bufs=1) as opool,
            tc.tile_pool(name="psum", bufs=1, space="PSUM") as ppool,
        ):
            # x loads ride the (otherwise idle) scalar engine's DMA queue so
            # they stream in parallel with the weight chunks on the sync queue
            xa_t = xpool.tile([PART, G, XROWS, XCOLS], bf, tag="xa")
            xb_t = xpool.tile([PART, G, XROWS, XCOLS], bf, tag="xb")
            nc.scalar.dma_start(xa_t[:, : G // 2], xa_d[:, :GH])
            nc.scalar.dma_start(xa_t[:, G // 2 :], xa_d[:, GH:])
            nc.scalar.dma_start(xb_t[:], xb_d[:])
            # per-kw weight chunks: [kh, 1(g-bcast), r, x]
            w_t = []
            for kw in range(K):
                wt = wpool.tile([PART, K, 1, RPO, W], bf, tag=f"w{kw}")
                nc.sync.dma_start(wt[:], wr_d[:, kw * KHRW : (kw + 1) * KHRW])
                w_t.append(wt)
            id_t = cpool.tile([PART, PART], bf, tag="id")
            nc.sync.dma_start(id_t[:], id_d[:])

            acc = [
                ppool.tile([PART, BANK], mybir.dt.float32, name=f"ps{j}", tag=f"ps{j}")
                for j in range(2 * RPO)
            ]
            ost = opool.tile([PART, OUTF], bf, tag="ost")

            last_kw = KW_ORDER[-1]
            for kw in KW_ORDER:
                if kw % 2 == 0:
                    xx, c0 = xa_t, kw
                else:
                    xx, c0 = xb_t, kw + 1
                for r in range(RPO):
                    # products for all 7 kh taps of this (kw, output-row r):
                    # tmp[p, kh, g, x] = x[p, g, r+kh, c0+x] * w[p, kw, kh, r, x]
                    # kw==0 is split into g-halves so compute can start after
                    # half of xa has landed; the final op is split so the
                    # first bank's evacuation overlaps the second half
                    split = kw == 0 or (kw == last_kw and r == RPO - 1)
                    halves = (0, 1) if split else (None,)
                    for hv in halves:
                        if hv is None:
                            gs, ge = 0, G
                        else:
                            gs, ge = hv * (G // 2), (hv + 1) * (G // 2)
                        ng = ge - gs
                        tmp = tpool.tile([PART, K, ng, W], bf, tag="tmp")
                        xap = xx[:, gs:ge, r : r + K, c0 : c0 + W].transpose(
                            [0, 2, 1, 3]
                        )
                        wap = w_t[kw][:, :, 0:1, r, :].broadcast_to((PART, K, ng, W))
                        nc.vector.tensor_mul(tmp[:], xap, wap)
                        # PE accumulates each kh-plane into PSUM; bank j = 2r+h
                        for kh in range(K):
                            for h in range(2):
                                if hv is not None and h != hv:
                                    continue
                                gofs = 0 if hv is not None else h * 8
                                j = 2 * r + h
                                nc.tensor.matmul(
                                    acc[j][:],
                                    id_t[:],
                                    tmp[:, kh, gofs : gofs + 8, :],
                                    start=(kw == 0 and kh == 0),
                                    stop=(kw == last_kw and kh == K - 1),
                                )
                        if kw == last_kw and hv is not None:
                            # evacuate this g-half's bank as soon as it stops
                            j = 2 * r + hv
                            nc.scalar.copy(
                                ost[:, j * BANK : (j + 1) * BANK], acc[j][:]
                            )
                    if kw == last_kw:
                        if r < RPO - 1:
                            for h in range(2):
                                j = 2 * r + h
                                nc.scalar.copy(
                                    ost[:, j * BANK : (j + 1) * BANK], acc[j][:]
                                )
                        nc.sync.dma_start(
                            out_d[:, r * 2 * BANK : (r + 1) * 2 * BANK],
                            ost[:, r * 2 * BANK : (r + 1) * 2 * BANK],
                        )

    _dedupe_ldweights(nc)
    nc.compile()
    return nc


def _dedupe_ldweights(nc):
    """All PE matmuls share one identity stationary; drop every InstLdweights
    after the first so the PE array keeps the loaded weights."""
    first_repr = None
    removed = 0
    for blk in nc.main_func.blocks:
        keep = []
        for inst in blk.instructions:
            if type(inst).__name__ == "InstLdweights":
                si = inst.sync_info
                clean = si is None or (not si.on_wait and not si.on_update)
                r = repr(inst.ins[0])
                if first_repr is None:
                    first_repr = r
                elif clean and r == first_repr:
                    removed += 1
                    continue
            keep.append(inst)
        blk.instructions[:] = keep
    return removed


def _prep_core(x, w, b, h):
    """Host-side shard prep for one core: pad/cast/re-layout x, slice w."""
    y0 = h * HHALF
    # padded half: rows y0-3 .. y0+34 (38), cols -3 .. 68 (72)
    xpad = np.zeros((C, HHALF + 2 * PAD, XCOLS), dtype=np.float32)
    rlo, rhi = y0 - PAD, y0 + HHALF + PAD
    slo, shi = max(rlo, 0), min(rhi, H)
    xpad[:, slo - rlo : shi - rlo, PAD : PAD + W] = x[b, :, slo:shi, :]
    # xa[p=(yo,cc), g, row, col] = xpad[g*16+cc, yo*4+row, col]
    v = np.lib.stride_tricks.sliding_window_view(xpad, XROWS, axis=1)
    v = v[:, ::RPO]  # [256, 8yo, 72col, 10row]
    v = v.reshape(G, WC, NYO, XCOLS, XROWS)
    xa = v.transpose(2, 1, 0, 4, 3).reshape(PART, G, XROWS, XCOLS)
    xb = np.zeros_like(xa)
    xb[:, :, :, 1:] = xa[:, :, :, :-1]
    # w[p=(yo,cc), kw, kh, r, x] = w[b,0,cc,kh*7+kw, yo*4+r, x]
    ws = w[b, 0, :, :, y0 : y0 + HHALF, :]  # [16cc, 49k, 32y, 64x]
    ws = ws.reshape(WC, K, K, NYO, RPO, W)  # [cc, kh, kw, yo, r, x]
    wr = ws.transpose(3, 0, 2, 1, 4, 5).reshape(PART, -1)  # [p, kw,kh,r,x]
    return (
        np.ascontiguousarray(xa).reshape(PART, -1).astype(_BF16),
        xb.reshape(PART, -1).astype(_BF16),
        np.ascontiguousarray(wr).astype(_BF16),
    )


def _unpack_core(o):
    """[128, 4096] bf16 -> [256, 32, 64] f32 for one core."""
    # o[p=(yo,cc)][j=(r,h)][8g, 64x]
    o = np.asarray(o, dtype=np.float32).reshape(NYO, WC, RPO, 2, 8, W)
    # channel = (h*8+g)*16+cc, row = yo*4+r
    return o.transpose(3, 4, 1, 0, 2, 5).reshape(C, HHALF, W)


def kernel(x: np.ndarray, weight: np.ndarray) -> np.ndarray:
    from concourse.bass_utils import run_bass_kernel_spmd

    if "nc" not in _cache:
        _cache["nc"] = _build()
    nc = _cache["nc"]

    in_maps = make_inmaps(x, weight)
    res = run_bass_kernel_spmd(nc, in_maps, list(range(NCORES)))

    out = np.empty((B, C, H, W), dtype=np.float32)
    for core in range(NCORES):
        b, h = core // 2, core % 2
        out[b, :, h * HHALF : (h + 1) * HHALF, :] = _unpack_core(
            res.results[core]["out"]
        )
    return out


def make_inmaps(x, weight):
    ident = np.eye(PART, dtype=_BF16)
    in_maps = []
    for core in range(NCORES):
        b, h = core // 2, core % 2
        xa, xb, wr = _prep_core(x, weight, b, h)
        in_maps.append({"xa": xa, "xb": xb, "wr": wr, "ident": ident})
    return in_maps


# revision 9
# speedup vs baseline: 1.3724x; 1.0950x over previous
import sys

for _p in ("/opt/trn_rl_repo", "/root/.axon_site/_ro/trn_rl_repo"):
    if _p not in sys.path:
        sys.path.insert(0, _p)

import numpy as np
import ml_dtypes

# Problem constants (nn_LocalConvolution): x [4,256,64,64] f32,
# weight [4,1,16,49,64,64] f32, K=7, pad=3, stride=1, dil=1.
# out[b, g*16+cc, y, x] = sum_k x_pad[b, g*16+cc, y+kh-3, x+kw-3] * w[b,0,cc,k,y,x]
#
# Sharding: 8 cores = 4 batches x 2 H-halves (32 rows each).
# Per-core layout: partition p = yo*16 + cc  (yo = y-octave 0..7 -> rows
# yo*4..yo*4+3 of the half;  cc = weight channel 0..15).  This keeps the
# weights UNREPLICATED across partitions (3.2MB/core instead of the 25.7MB
# a channel-major layout needs), at the cost of a 2.5x row-halo on x.
B, C, H, W = 4, 256, 64, 64
WC, K, PAD = 16, 7, 3
NCORES = 8
HHALF = H // 2          # 32 output rows per core
PART = 128
G = C // WC             # 16 groups sharing each weight channel
NYO = 8                 # y-octaves per half (4 rows each)
RPO = HHALF // NYO      # 4 rows per octave
XROWS = RPO + K - 1     # 10 input rows per octave (3 halo each side)
XCOLS = 72              # 64 + 3 left pad + 5 right pad (even alignment)
OUTF = G * RPO * W      # 4096 f32 per partition = all 8 PSUM banks
BANK = 512

# xa-based (even) kw first so xb's DMA has ~60us of slack to land
KW_ORDER = [0, 2, 4, 6, 1, 3, 5]

_BF16 = ml_dtypes.bfloat16
_cache = {}


def _build():
    import concourse.bacc as bacc
    import concourse.mybir as mybir
    import concourse.tile as tile

    nc = bacc.Bacc(None, target_bir_lowering=False)
    bf = mybir.dt.bfloat16

    xa_d = nc.dram_tensor("xa", (PART, G * XROWS * XCOLS), bf, kind="ExternalInput")
    xb_d = nc.dram_tensor("xb", (PART, G * XROWS * XCOLS), bf, kind="ExternalInput")
    wr_d = nc.dram_tensor("wr", (PART, K * K * RPO * W), bf, kind="ExternalInput")
    id_d = nc.dram_tensor("ident", (PART, PART), bf, kind="ExternalInput")
    out_d = nc.dram_tensor("out", (PART, OUTF), bf, kind="ExternalOutput")

    KHRW = K * RPO * W  # 1792 weight elems per kw-chunk per partition
    GH = G * XROWS * XCOLS // 2  # half of xa's free size

    with tile.TileContext(nc) as tc:
        with (
            tc.tile_pool(name="xpool", bufs=1) as xpool,
            tc.tile_pool(name="cpool", bufs=1) as cpool,
            tc.tile_pool(name="wpool", bufs=1) as wpool,
            tc.tile_pool(name="tpool", bufs=5) as tpool,
            tc.tile_pool(name="opool", bufs=1) as opool,
            tc.tile_pool(name="psum", bufs=1, space="PSUM") as ppool,
        ):
            # x loads ride the (otherwise idle) scalar engine's DMA queue so
            # they stream in parallel with the weight chunks on the sync queue
            xa_t = xpool.tile([PART, G, XROWS, XCOLS], bf, tag="xa")
            xb_t = xpool.tile([PART, G, XROWS, XCOLS], bf, tag="xb")
            GQ = G * XROWS * XCOLS // 4  # xa quarter (4 g's)
            for q in range(4):
                nc.scalar.dma_start(
                    xa_t[:, 4 * q : 4 * (q + 1)], xa_d[:, q * GQ : (q + 1) * GQ]
                )
            nc.scalar.dma_start(xb_t[:], xb_d[:])
            id_t = cpool.tile([PART, PART], bf, tag="id")
            nc.sync.dma_start(id_t[:], id_d[:])
            # per-kw weight chunks: [kh, 1(g-bcast), r, x]
            w_t = []
            for kw in range(K):
                wt = wpool.tile([PART, K, 1, RPO, W], bf, tag=f"w{kw}")
                nc.sync.dma_start(wt[:], wr_d[:, kw * KHRW : (kw + 1) * KHRW])
                w_t.append(wt)

            acc = [
                ppool.tile([PART, BANK], mybir.dt.float32, name=f"ps{j}", tag=f"ps{j}")
                for j in range(2 * RPO)
            ]
            ost = opool.tile([PART, OUTF], bf, tag="ost")

            last_kw = KW_ORDER[-1]

            def emit_op(kw, r, gs, ng):
                # products for the 7 kh taps of (kw, output-row r), groups
                # gs..gs+ng: tmp[p,kh,g,x] = x[p,g,r+kh,c0+x] * w[p,kw,kh,r,x]
                if kw % 2 == 0:
                    xx, c0 = xa_t, kw
                else:
                    xx, c0 = xb_t, kw + 1
                tmp = tpool.tile([PART, K, ng, W], bf, tag="tmp")
                xap = xx[:, gs : gs + ng, r : r + K, c0 : c0 + W].transpose(
                    [0, 2, 1, 3]
                )
                wap = w_t[kw][:, :, 0:1, r, :].broadcast_to((PART, K, ng, W))
                nc.vector.tensor_mul(tmp[:], xap, wap)
                # PE accumulates each kh-plane into PSUM; bank j = 2r + g-half,
                # 256-col sub-bank slices when ng==4 (has_written is per
                # element, so partial-bank start flags are fine)
                for kh in range(K):
                    for gofs in range(0, ng, 8 if ng >= 8 else 4):
                        nw = min(8, ng)
                        g0 = gs + gofs
                        j = 2 * r + g0 // 8
                        c = (g0 % 8) * 64
                        # start=True clears has_written for the WHOLE bank, so
                        # only the first quarter touching a bank may set it;
                        # later partial writes overwrite-vs-accumulate via the
                        # per-element has_written bits
                        nc.tensor.matmul(
                            acc[j][:, c : c + nw * 64],
                            id_t[:],
                            tmp[:, kh, gofs : gofs + nw, :],
                            start=(kw == 0 and kh == 0 and g0 % 8 == 0),
                            stop=(kw == last_kw and kh == K - 1),
                            skip_group_check=True,
                        )

            def evac(j):
                nc.scalar.copy(ost[:, j * BANK : (j + 1) * BANK], acc[j][:])

            # kw=0 in g-quarters, ordered so each quarter's ops start right
            # after its slice of xa lands
            for gq in range(4):
                for r in range(RPO):
                    emit_op(0, r, 4 * gq, 4)
            for kw in KW_ORDER[1:]:
                for r in range(RPO):
                    if kw == last_kw and r == RPO - 1:
                        # final op split so bank 6 evacuates under the h1 half
                        emit_op(kw, r, 0, 8)
                        evac(2 * r)
                        emit_op(kw, r, 8, 8)
                        evac(2 * r + 1)
                    else:
                        emit_op(kw, r, 0, G)
                        if kw == last_kw:
                            evac(2 * r)
                            evac(2 * r + 1)
                    if kw == last_kw:
                        nc.sync.dma_start(
                            out_d[:, r * 2 * BANK : (r + 1) * 2 * BANK],
                            ost[:, r * 2 * BANK : (r + 1) * 2 * BANK],
                        )

    _dedupe_ldweights(nc)
    nc.compile()
    return nc


def _dedupe_ldweights(nc):
    """All PE matmuls share one identity stationary; drop every InstLdweights
    after the first so the PE array keeps the loaded weights."""
    first_repr = None
    removed = 0
    for blk in nc.main_func.blocks:
        keep = []
        for inst in blk.instructions:
            if type(inst).__name__ == "InstLdweights":
                si = inst.sync_info
                clean = si is None or (not si.on_wait and not si.on_update)
                r = repr(inst.ins[0])
                if first_repr is None:
                    first_repr = r
                elif clean and r == first_repr:
                    removed += 1
                    continue
            keep.append(inst)
        blk.instructions[:] = keep
    return removed


def _prep_core(x, w, b, h):
    """Host-side shard prep for one core: pad/cast/re-layout x, slice w."""
    y0 = h * HHALF
    # padded half: rows y0-3 .. y0+34 (38), cols -3 .. 68 (72)
    xpad = np.zeros((C, HHALF + 2 * PAD, XCOLS), dtype=np.float32)
    rlo, rhi = y0 - PAD, y0 + HHALF + PAD
    slo, shi = max(rlo, 0), min(rhi, H)
    xpad[:, slo - rlo : shi - rlo, PAD : PAD + W] = x[b, :, slo:shi, :]
    # xa[p=(yo,cc), g, row, col] = xpad[g*16+cc, yo*4+row, col]
    v = np.lib.stride_tricks.sliding_window_view(xpad, XROWS, axis=1)
    v = v[:, ::RPO]  # [256, 8yo, 72col, 10row]
    v = v.reshape(G, WC, NYO, XCOLS, XROWS)
    xa = v.transpose(2, 1, 0, 4, 3).reshape(PART, G, XROWS, XCOLS)
    xb = np.zeros_like(xa)
    xb[:, :, :, 1:] = xa[:, :, :, :-1]
    # w[p=(yo,cc), kw, kh, r, x] = w[b,0,cc,kh*7+kw, yo*4+r, x]
    ws = w[b, 0, :, :, y0 : y0 + HHALF, :]  # [16cc, 49k, 32y, 64x]
    ws = ws.reshape(WC, K, K, NYO, RPO, W)  # [cc, kh, kw, yo, r, x]
    wr = ws.transpose(3, 0, 2, 1, 4, 5).reshape(PART, -1)  # [p, kw,kh,r,x]
    return (
        np.ascontiguousarray(xa).reshape(PART, -1).astype(_BF16),
        xb.reshape(PART, -1).astype(_BF16),
        np.ascontiguousarray(wr).astype(_BF16),
    )


def _unpack_core(o):
    """[128, 4096] bf16 -> [256, 32, 64] f32 for one core."""
    # o[p=(yo,cc)][j=(r,h)][8g, 64x]
    o = np.asarray(o, dtype=np.float32).reshape(NYO, WC, RPO, 2, 8, W)
    # channel = (h*8+g)*16+cc, row = yo*4+r
    return o.transpose(3, 4, 1, 0, 2, 5).reshape(C, HHALF, W)


def kernel(x: np.ndarray, weight: np.ndarray) -> np.ndarray:
    from concourse.bass_utils import run_bass_kernel_spmd

    if "nc" not in _cache:
        _cache["nc"] = _build()
    nc = _cache["nc"]

    in_maps = make_inmaps(x, weight)
    res = run_bass_kernel_spmd(nc, in_maps, list(range(NCORES)))

    out = np.empty((B, C, H, W), dtype=np.float32)
    for core in range(NCORES):
        b, h = core // 2, core % 2
        out[b, :, h * HHALF : (h + 1) * HHALF, :] = _unpack_core(
            res.results[core]["out"]
        )
    return out


def make_inmaps(x, weight):
    ident = np.eye(PART, dtype=_BF16)
    in_maps = []
    for core in range(NCORES):
        b, h = core // 2, core % 2
        xa, xb, wr = _prep_core(x, weight, b, h)
        in_maps.append({"xa": xa, "xb": xb, "wr": wr, "ident": ident})
    return in_maps


# revision 12
# speedup vs baseline: 1.3761x; 1.0027x over previous
import sys

for _p in ("/opt/trn_rl_repo", "/root/.axon_site/_ro/trn_rl_repo"):
    if _p not in sys.path:
        sys.path.insert(0, _p)

import numpy as np
import ml_dtypes

# Problem constants (nn_LocalConvolution): x [4,256,64,64] f32,
# weight [4,1,16,49,64,64] f32, K=7, pad=3, stride=1, dil=1.
# out[b, g*16+cc, y, x] = sum_k x_pad[b, g*16+cc, y+kh-3, x+kw-3] * w[b,0,cc,k,y,x]
#
# Sharding: 8 cores = 4 batches x 2 H-halves (32 rows each).
# Per-core layout: partition p = yo*16 + cc  (yo = y-octave 0..7 -> rows
# yo*4..yo*4+3 of the half;  cc = weight channel 0..15).  This keeps the
# weights UNREPLICATED across partitions (3.2MB/core instead of the 25.7MB
# a channel-major layout needs), at the cost of a 2.5x row-halo on x.
B, C, H, W = 4, 256, 64, 64
WC, K, PAD = 16, 7, 3
NCORES = 8
HHALF = H // 2          # 32 output rows per core
PART = 128
G = C // WC             # 16 groups sharing each weight channel
NYO = 8                 # y-octaves per half (4 rows each)
RPO = HHALF // NYO      # 4 rows per octave
XROWS = RPO + K - 1     # 10 input rows per octave (3 halo each side)
XCOLS = 72              # 64 + 3 left pad + 5 right pad (even alignment)
OUTF = G * RPO * W      # 4096 f32 per partition = all 8 PSUM banks
BANK = 512

# xa-based (even) kw first so xb's DMA has ~60us of slack to land
KW_ORDER = [0, 2, 4, 6, 1, 3, 5]
# g-group sizes for the xa DMA slices / kw=0 ops (small first for fast start)
XA_SPLITS = [2, 2, 4, 4, 4]

_BF16 = ml_dtypes.bfloat16
_cache = {}


def _build():
    import concourse.bacc as bacc
    import concourse.mybir as mybir
    import concourse.tile as tile

    nc = bacc.Bacc(None, target_bir_lowering=False)
    bf = mybir.dt.bfloat16

    xa_d = nc.dram_tensor("xa", (PART, G * XROWS * XCOLS), bf, kind="ExternalInput")
    xb_d = nc.dram_tensor("xb", (PART, G * XROWS * XCOLS), bf, kind="ExternalInput")
    wr_d = nc.dram_tensor("wr", (PART, K * K * RPO * W), bf, kind="ExternalInput")
    id_d = nc.dram_tensor("ident", (PART, PART), bf, kind="ExternalInput")
    out_d = nc.dram_tensor("out", (PART, OUTF), bf, kind="ExternalOutput")

    KHRW = K * RPO * W  # 1792 weight elems per kw-chunk per partition
    GH = G * XROWS * XCOLS // 2  # half of xa's free size

    with tile.TileContext(nc) as tc:
        with (
            tc.tile_pool(name="xpool", bufs=1) as xpool,
            tc.tile_pool(name="cpool", bufs=1) as cpool,
            tc.tile_pool(name="wpool", bufs=1) as wpool,
            tc.tile_pool(name="tpool", bufs=5) as tpool,
            tc.tile_pool(name="opool", bufs=1) as opool,
            tc.tile_pool(name="psum", bufs=1, space="PSUM") as ppool,
        ):
            # x loads ride the (otherwise idle) scalar engine's DMA queue so
            # they stream in parallel with the weight chunks on the sync queue
            xa_t = xpool.tile([PART, G, XROWS, XCOLS], bf, tag="xa")
            xb_t = xpool.tile([PART, G, XROWS, XCOLS], bf, tag="xb")
            # first slices small so the very first op can start ~10us in
            GF = XROWS * XCOLS  # free elems per g
            gofs = 0
            for ng in XA_SPLITS:
                nc.scalar.dma_start(
                    xa_t[:, gofs : gofs + ng],
                    xa_d[:, gofs * GF : (gofs + ng) * GF],
                )
                gofs += ng
            nc.scalar.dma_start(xb_t[:], xb_d[:])
            id_t = cpool.tile([PART, PART], bf, tag="id")
            nc.sync.dma_start(id_t[:], id_d[:])
            # per-kw weight chunks: [kh, 1(g-bcast), r, x]
            w_t = []
            for kw in range(K):
                wt = wpool.tile([PART, K, 1, RPO, W], bf, tag=f"w{kw}")
                nc.sync.dma_start(wt[:], wr_d[:, kw * KHRW : (kw + 1) * KHRW])
                w_t.append(wt)

            acc = [
                ppool.tile([PART, BANK], mybir.dt.float32, name=f"ps{j}", tag=f"ps{j}")
                for j in range(2 * RPO)
            ]
            ost = opool.tile([PART, OUTF], bf, tag="ost")

            last_kw = KW_ORDER[-1]

            def emit_op(kw, r, gs, ng):
                # products for the 7 kh taps of (kw, output-row r), groups
                # gs..gs+ng: tmp[p,kh,g,x] = x[p,g,r+kh,c0+x] * w[p,kw,kh,r,x]
                if kw % 2 == 0:
                    xx, c0 = xa_t, kw
                else:
                    xx, c0 = xb_t, kw + 1
                tmp = tpool.tile([PART, K, ng, W], bf, tag="tmp")
                xap = xx[:, gs : gs + ng, r : r + K, c0 : c0 + W].transpose(
                    [0, 2, 1, 3]
                )
                wap = w_t[kw][:, :, 0:1, r, :].broadcast_to((PART, K, ng, W))
                nc.vector.tensor_mul(tmp[:], xap, wap)
                # PE accumulates each kh-plane into PSUM; bank j = 2r + g-half,
                # 256-col sub-bank slices when ng==4 (has_written is per
                # element, so partial-bank start flags are fine)
                for kh in range(K):
                    for gofs in range(0, ng, 8 if ng >= 8 else 4):
                        nw = min(8, ng)
                        g0 = gs + gofs
                        j = 2 * r + g0 // 8
                        c = (g0 % 8) * 64
                        # start=True clears has_written for the WHOLE bank, so
                        # only the first quarter touching a bank may set it;
                        # later partial writes overwrite-vs-accumulate via the
                        # per-element has_written bits
                        nc.tensor.matmul(
                            acc[j][:, c : c + nw * 64],
                            id_t[:],
                            tmp[:, kh, gofs : gofs + nw, :],
                            start=(kw == 0 and kh == 0 and g0 % 8 == 0),
                            stop=(kw == last_kw and kh == K - 1),
                            skip_group_check=True,
                        )

            def evac(j):
                nc.scalar.copy(ost[:, j * BANK : (j + 1) * BANK], acc[j][:])

            # kw=0 in g-slices, ordered so each slice's ops start right
            # after its piece of xa lands
            gofs = 0
            for ng in XA_SPLITS:
                for r in range(RPO):
                    emit_op(0, r, gofs, ng)
                gofs += ng
            for kw in KW_ORDER[1:]:
                for r in range(RPO):
                    if kw == last_kw and r == RPO - 1:
                        # final op in quarters so evacuations overlap compute
                        emit_op(kw, r, 0, 4)
                        emit_op(kw, r, 4, 4)
                        evac(2 * r)
                        emit_op(kw, r, 8, 4)
                        emit_op(kw, r, 12, 4)
                        evac(2 * r + 1)
                    else:
                        emit_op(kw, r, 0, G)
                        if kw == last_kw:
                            evac(2 * r)
                            evac(2 * r + 1)
                    if kw == last_kw:
                        nc.sync.dma_start(
                            out_d[:, r * 2 * BANK : (r + 1) * 2 * BANK],
                            ost[:, r * 2 * BANK : (r + 1) * 2 * BANK],
                        )

    _dedupe_ldweights(nc)
    nc.compile()
    return nc


def _dedupe_ldweights(nc):
    """All PE matmuls share one identity stationary; drop every InstLdweights
    after the first so the PE array keeps the loaded weights."""
    first_repr = None
    removed = 0
    for blk in nc.main_func.blocks:
        keep = []
        for inst in blk.instructions:
            if type(inst).__name__ == "InstLdweights":
                si = inst.sync_info
                clean = si is None or (not si.on_wait and not si.on_update)
                r = repr(inst.ins[0])
                if first_repr is None:
                    first_repr = r
                elif clean and r == first_repr:
                    removed += 1
                    continue
            keep.append(inst)
        blk.instructions[:] = keep
    return removed


def _prep_core(x, w, b, h):
    """Host-side shard prep for one core: pad/cast/re-layout x, slice w."""
    y0 = h * HHALF
    # padded half: rows y0-3 .. y0+34 (38), cols -3 .. 68 (72)
    xpad = np.zeros((C, HHALF + 2 * PAD, XCOLS), dtype=np.float32)
    rlo, rhi = y0 - PAD, y0 + HHALF + PAD
    slo, shi = max(rlo, 0), min(rhi, H)
    xpad[:, slo - rlo : shi - rlo, PAD : PAD + W] = x[b, :, slo:shi, :]
    # xa[p=(yo,cc), g, row, col] = xpad[g*16+cc, yo*4+row, col]
    v = np.lib.stride_tricks.sliding_window_view(xpad, XROWS, axis=1)
    v = v[:, ::RPO]  # [256, 8yo, 72col, 10row]
    v = v.reshape(G, WC, NYO, XCOLS, XROWS)
    xa = v.transpose(2, 1, 0, 4, 3).reshape(PART, G, XROWS, XCOLS)
    xb = np.zeros_like(xa)
    xb[:, :, :, 1:] = xa[:, :, :, :-1]
    # w[p=(yo,cc), kw, kh, r, x] = w[b,0,cc,kh*7+kw, yo*4+r, x]
    ws = w[b, 0, :, :, y0 : y0 + HHALF, :]  # [16cc, 49k, 32y, 64x]
    ws = ws.reshape(WC, K, K, NYO, RPO, W)  # [cc, kh, kw, yo, r, x]
    wr = ws.transpose(3, 0, 2, 1, 4, 5).reshape(PART, -1)  # [p, kw,kh,r,x]
    return (
        np.ascontiguousarray(xa).reshape(PART, -1).astype(_BF16),
        xb.reshape(PART, -1).astype(_BF16),
        np.ascontiguousarray(wr).astype(_BF16),
    )


def _unpack_core(o):
    """[128, 4096] bf16 -> [256, 32, 64] f32 for one core."""
    # o[p=(yo,cc)][j=(r,h)][8g, 64x]
    o = np.asarray(o, dtype=np.float32).reshape(NYO, WC, RPO, 2, 8, W)
    # channel = (h*8+g)*16+cc, row = yo*4+r
    return o.transpose(3, 4, 1, 0, 2, 5).reshape(C, HHALF, W)


def kernel(x: np.ndarray, weight: np.ndarray) -> np.ndarray:
    from concourse.bass_utils import run_bass_kernel_spmd

    if "nc" not in _cache:
        _cache["nc"] = _build()
    nc = _cache["nc"]

    in_maps = make_inmaps(x, weight)
    res = run_bass_kernel_spmd(nc, in_maps, list(range(NCORES)))

    out = np.empty((B, C, H, W), dtype=np.float32)
    for core in range(NCORES):
        b, h = core // 2, core % 2
        out[b, :, h * HHALF : (h + 1) * HHALF, :] = _unpack_core(
            res.results[core]["out"]
        )
    return out


def make_inmaps(x, weight):
    ident = np.eye(PART, dtype=_BF16)
    in_maps = []
    for core in range(NCORES):
        b, h = core // 2, core % 2
        xa, xb, wr = _prep_core(x, weight, b, h)
        in_maps.append({"xa": xa, "xb": xb, "wr": wr, "ident": ident})
    return in_maps
